# revision 4
# baseline (speedup 1.0000x reference)
"""Trainium2 Bass kernel for nn_AttentionHeadless (sparse_attention).

Reference computation (B=2, Q=512, K=512, T=256):
    k = key @ Wk.T; q = query @ Wq.T; v = value @ Wva.T
    logits[b,kk,q,u] = sum_t Wal[u,t] * k[b,kk,t] * q[b,q,t]
    scale = swishmax(logits, axis=-2)      # normalize over Q
    out = (sum_kk v[b,kk] * scale) @ Wvo.T

Sharding: data-parallel over (b, kk): each of 8 cores takes 64 of the 512
K-rows per batch; partial value-sums commute with the final Wvo matmul, so
each core emits a partial [B, T, Q] output; host sums 8 partials + Wvo.

Per-core pipeline, layout [u on 128 partitions (2 chunks uc), q free].
With y = L*exp(L-M) and E = exp(L-M), swishmax denominator is
    den = sum_q|y| + max_q E          (exactly, for shift M)
so no max-recovery/Newton pass is needed: max_q E comes from a 4x-mode
tensor_scalar max-accumulate over E, and sum_q|y| from relu identities:
    uc0: sum|y| = 2*sum(relu(y)) - sum(y)   (sum(y) free from the fused
         multiply's add-accumulator)
    uc1: sum|y| = sum(relu(y)) - sum(min(y,0))
Engine split per k-row (cost-model ns):
    PE   main matmul fp16 (853) + diag-accumulate fp16 (427)
    ACT  E = Exp(lps) [both uc] (1038) + Lc = Copy(lps-uc1) fp16 (612)
    DVE  walk (253), y0 = lps*E custom mul w/ sum-accum (658),
         relu+(y0), maxE0, relu+(y1), relu-(y1), maxE1 (5 x 194), smalls
    GPS  y1 = Lc*E1 tensor_mul (1111) + diag builds via broadcast mul (698)
GPSIMD cannot touch PSUM, hence the ACT fp16 copy of the uc1 logits.
"""

import numpy as np
import ml_dtypes

import concourse.bacc as bacc
import concourse.mybir as mybir
import concourse.tile as tile
from concourse import dve_ops
from concourse.bass_utils import run_bass_kernel_spmd
from concourse.dve_spec import Spec, Src0, Src1, AluOp, lower as _uop_lower
from concourse.dve_uop import DveOpSpec

B, Q, K, T = 2, 512, 512, 256
NCORES = 8
KSH = K // NCORES  # 64 K-rows per core per batch
BATCH = 16
MSHIFT = 3.0
P = 128

f32 = mybir.dt.float32
f32r = mybir.dt.float32r
fp16 = mybir.dt.float16
AF = mybir.ActivationFunctionType
ALU = mybir.AluOpType


def _register_dve_op(name, spec, subdim=False):
    for op in dve_ops.OPS:
        if op.name == name:
            return op
    shas = {}
    for ver in ("v3", "v4"):
        try:
            uops = _uop_lower(spec, ver=ver)
            shas[ver] = DveOpSpec(name=name, uops=uops).sha(ver)
        except Exception:
            pass
    op = dve_ops.DveOp(name, spec, subdim=subdim, uops_sha=shas)
    dve_ops.OPS.append(op)
    dve_ops._SUB_OPCODE_FOR_NAME[name] = (
        dve_ops._CUSTOM_DVE_ROW_BASE + len(dve_ops.OPS) - 1
    )
    dve_ops.CUSTOM_DVE_SPECS[name] = spec
    return op


def _ref_mul_addacc(in0, in1, c0, c1, c2):
    b = (in0.astype(np.float32) * in1.astype(np.float32)).astype(np.float32)
    return b, b.reshape(b.shape[0], -1).sum(axis=-1, keepdims=True)


MUL_ADDACC = _register_dve_op(
    "MUL_ADDACC_ANT",
    Spec(body=Src0 * Src1, accum=AluOp.ADD, reference=_ref_mul_addacc),
)

# kept for compatibility with older helper scripts
def _ref_mul_maxacc(in0, in1, c0, c1, c2):
    b = (in0.astype(np.float32) * in1.astype(np.float32)).astype(np.float32)
    return b, b.reshape(b.shape[0], -1).max(axis=-1, keepdims=True)


MUL_MAXACC = _register_dve_op(
    "MUL_MAXACC_ANT",
    Spec(body=Src0 * Src1, accum=AluOp.MAX, reference=_ref_mul_maxacc),
)


def build(n_cores=NCORES):
    nc = bacc.Bacc("TRN2", target_bir_lowering=False, debug=False, num_devices=n_cores)

    # ---- DRAM I/O (per-core) ----
    d_wqT = nc.dram_tensor("wqT", [T, T], f32r, kind="ExternalInput").ap()
    d_wkT = nc.dram_tensor("wkT", [T, T], f32r, kind="ExternalInput").ap()
    d_wvaT = nc.dram_tensor("wvaT", [T, T], f32r, kind="ExternalInput").ap()
    d_walT = nc.dram_tensor("walT", [T, T], fp16, kind="ExternalInput").ap()
    d_qT = nc.dram_tensor("qT", [B, T, Q], f32r, kind="ExternalInput").ap()
    d_keyT = nc.dram_tensor("keyT", [B, T, KSH], f32r, kind="ExternalInput").ap()
    d_valT = nc.dram_tensor("valT", [B, T, KSH], f32r, kind="ExternalInput").ap()
    d_eye = nc.dram_tensor("eye", [P, P], fp16, kind="ExternalInput").ap()
    d_out = nc.dram_tensor("outT", [B, T, Q], f32, kind="ExternalOutput").ap()

    NB = KSH // BATCH  # batches per b

    with tile.TileContext(nc) as tc:
        cpool = tc.alloc_tile_pool(name="consts", bufs=1)
        lps_pool = tc.alloc_tile_pool(name="lps", bufs=3, space="PSUM")
        acc_pool = tc.alloc_tile_pool(name="accp", bufs=1, space="PSUM")
        walk_pool = tc.alloc_tile_pool(name="walk", bufs=8)
        e_pool = tc.alloc_tile_pool(name="epool", bufs=5)
        lc_pool = tc.alloc_tile_pool(name="lcpool", bufs=5)
        y_pool = tc.alloc_tile_pool(name="ypool", bufs=2)
        red_pool = tc.alloc_tile_pool(name="red", bufs=8)
        sm_pool = tc.alloc_tile_pool(name="smalls", bufs=3)
        diag_pool = tc.alloc_tile_pool(name="diag", bufs=8)
        scrap_pool = tc.alloc_tile_pool(name="scrap", bufs=6)
        out_pool = tc.alloc_tile_pool(name="outp", bufs=2)

        # ---- load constants ----
        wqT = cpool.tile([P, 2, T], f32r, tag="wqT")
        wkT = cpool.tile([P, 2, T], f32r, tag="wkT")
        wvaT = cpool.tile([P, 2, T], f32r, tag="wvaT")
        walT = cpool.tile([P, 2, T], fp16, tag="walT")
        eye = cpool.tile([P, P], fp16, tag="eye")
        qT = cpool.tile([P, B, 2, Q], f32r, tag="qT")
        keyT = cpool.tile([P, B, 2, KSH], f32r, tag="keyT")
        valT = cpool.tile([P, B, 2, KSH], f32r, tag="valT")
        for w_sb, w_d in ((wqT, d_wqT), (wkT, d_wkT), (wvaT, d_wvaT), (walT, d_walT)):
            for sc in range(2):
                nc.sync.dma_start(w_sb[:, sc, :], w_d[sc * P : (sc + 1) * P, :])
        nc.sync.dma_start(eye[:], d_eye)
        for b in range(B):
            for sc in range(2):
                nc.sync.dma_start(qT[:, b, sc, :], d_qT[b, sc * P : (sc + 1) * P, :])
                nc.sync.dma_start(keyT[:, b, sc, :], d_keyT[b, sc * P : (sc + 1) * P, :])
                nc.sync.dma_start(valT[:, b, sc, :], d_valT[b, sc * P : (sc + 1) * P, :])

        biasM = cpool.tile([P, 1], f32, tag="biasM")
        nc.vector.memset(biasM[:], -MSHIFT)

        # ---- projections ----
        qpT = cpool.tile([P, B, 2, Q], fp16, tag="qpT")
        kp = cpool.tile([P, B, 2, KSH], f32, tag="kp")
        vp = cpool.tile([P, B, 2, KSH], f32, tag="vp")
        for b in range(B):
            ps = lps_pool.tile([P, 2, Q], f32, tag="lps")
            for t_c in range(2):
                for sc in range(2):
                    nc.tensor.matmul(
                        ps[:, t_c, :],
                        wqT[:, sc, t_c * P : (t_c + 1) * P],
                        qT[:, b, sc, :],
                        start=(sc == 0),
                        stop=(sc == 1),
                    )
            nc.scalar.copy(qpT[:, b, :, :], ps[:, :, :])
            pskv = lps_pool.tile([P, 2, 2, KSH], f32, tag="lps")
            for t_c in range(2):
                for sc in range(2):
                    nc.tensor.matmul(
                        pskv[:, 0, t_c, :],
                        wkT[:, sc, t_c * P : (t_c + 1) * P],
                        keyT[:, b, sc, :],
                        start=(sc == 0),
                        stop=(sc == 1),
                    )
            for t_c in range(2):
                for sc in range(2):
                    nc.tensor.matmul(
                        pskv[:, 1, t_c, :],
                        wvaT[:, sc, t_c * P : (t_c + 1) * P],
                        valT[:, b, sc, :],
                        start=(sc == 0),
                        stop=(sc == 1),
                    )
            nc.scalar.copy(kp[:, b, :, :], pskv[:, 0, :, :])
            nc.scalar.copy(vp[:, b, :, :], pskv[:, 1, :, :])

        # ---- main loop ----
        for b in range(B):
            acc = acc_pool.tile([P, 2, Q], f32, tag="acc")
            pending = None

            def acc_pair(pend, j):
                py, pcc, pbatch = pend
                for uc in range(2):
                    diagt = diag_pool.tile([P, P], fp16, tag="diagt")
                    nc.gpsimd.tensor_mul(
                        diagt[:], eye[:], pcc[:, uc, j : j + 1].broadcast_to([P, P])
                    )
                    nc.tensor.matmul(
                        acc[:, uc, :],
                        diagt[:],
                        py[:, j, uc, :],
                        start=(pbatch == 0 and j == 0),
                        stop=(pbatch == NB - 1 and j == BATCH - 1),
                        skip_group_check=True,
                    )

            for batch in range(NB):
                yring = y_pool.tile([P, BATCH, 2, Q], fp16, tag="yring")
                sy0 = red_pool.tile([P, BATCH], f32, tag="sy0")
                sp = red_pool.tile([P, 2, BATCH], f32, tag="sp")
                sm1 = red_pool.tile([P, BATCH], f32, tag="sm1")
                me = red_pool.tile([P, 2, BATCH], f32, tag="me")
                for j in range(BATCH):
                    kk = batch * BATCH + j
                    if pending is not None:
                        acc_pair(pending, j)
                    walk = walk_pool.tile([P, 2, T], fp16, tag="walk")
                    for t_c in range(2):
                        nc.vector.tensor_scalar_mul(
                            walk[:, t_c, :], walT[:, t_c, :], kp[:, b, t_c, kk : kk + 1]
                        )
                    lps = lps_pool.tile([P, 2, Q], f32, tag="lps")
                    for uc in range(2):
                        for t_c in range(2):
                            nc.tensor.matmul(
                                lps[:, uc, :],
                                walk[:, t_c, uc * P : (uc + 1) * P],
                                qpT[:, b, t_c, :],
                                start=(t_c == 0),
                                stop=(t_c == 1),
                            )
                    E = e_pool.tile([P, 2, Q], fp16, tag="E")
                    _ei = nc.scalar.activation(
                        E[:, :, :], lps[:, :, :], AF.Exp, bias=biasM[:], scale=1.0
                    )
                    _ei.ins.bass_priority = -50
                    Lc = lc_pool.tile([P, Q], fp16, tag="Lc")
                    _ci = nc.scalar.activation(Lc[:], lps[:, 1, :], AF.Copy, bias=0.0, scale=1.0)
                    _ci.ins.bass_priority = -49

                    # uc0: fused multiply on DVE with sum-accumulator
                    _di = nc.vector._custom_dve(
                        MUL_ADDACC,
                        out=yring[:, j, 0, :],
                        in0=lps[:, 0, :],
                        in1=E[:, 0, :],
                        accum_out=sy0[:, j : j + 1],
                    )
                    _di.ins.bass_priority = -40
                    # uc1: multiply on GPSIMD from the SBUF fp16 logits copy
                    _gi = nc.gpsimd.tensor_mul(yring[:, j, 1, :], Lc[:], E[:, 1, :])
                    _gi.ins.bass_priority = -45

                    # reductions (all 4x-mode tensor_scalar)
                    for uc in range(2):
                        scr = scrap_pool.tile([P, Q], fp16, tag="scr")
                        nc.vector.tensor_scalar(
                            scr[:],
                            yring[:, j, uc, :],
                            0.0,
                            None,
                            op0=ALU.max,
                            op1=ALU.add,
                            accum_out=sp[:, uc, j : j + 1],
                        )
                        scrE = scrap_pool.tile([P, Q], fp16, tag="scrE")
                        nc.vector.tensor_scalar(
                            scrE[:],
                            E[:, uc, :],
                            1.0,
                            None,
                            op0=ALU.mult,
                            op1=ALU.max,
                            accum_out=me[:, uc, j : j + 1],
                        )
                    scrm = scrap_pool.tile([P, Q], fp16, tag="scrm")
                    nc.vector.tensor_scalar(
                        scrm[:],
                        yring[:, j, 1, :],
                        0.0,
                        None,
                        op0=ALU.min,
                        op1=ALU.add,
                        accum_out=sm1[:, j : j + 1],
                    )

                # ---- batched smalls: den and c ----
                den = sm_pool.tile([P, 2, BATCH], f32, tag="den")
                nc.vector.tensor_scalar_mul(den[:, 0, :], sp[:, 0, :], 2.0)
                nc.vector.tensor_sub(den[:, 0, :], den[:, 0, :], sy0[:, :])
                nc.vector.tensor_sub(den[:, 1, :], sp[:, 1, :], sm1[:, :])
                nc.vector.tensor_add(den[:, :, :], den[:, :, :], me[:, :, :])
                nc.vector.reciprocal_approx_fast(den[:, :, :], den[:, :, :])
                cc = sm_pool.tile([P, 2, BATCH], f32, tag="cc")
                nc.vector.tensor_mul(
                    cc[:, :, :], den[:, :, :],
                    vp[:, b, :, batch * BATCH : (batch + 1) * BATCH],
                )
                pending = (yring, cc, batch)

            for j in range(BATCH):
                acc_pair(pending, j)

            # ---- drain b: DMA the accumulated value-sum partial ----
            st = out_pool.tile([P, 2, Q], f32, tag="st")
            nc.scalar.copy(st[:, :, :], acc[:, :, :])
            for sc in range(2):
                nc.sync.dma_start(d_out[b, sc * P : (sc + 1) * P, :], st[:, sc, :])

        for pl in (out_pool, scrap_pool, diag_pool, sm_pool, red_pool, y_pool,
                   lc_pool, e_pool, walk_pool, acc_pool, lps_pool, cpool):
            pl.release()

    nc.compile()
    return nc


_NC_CACHE = {}


def _get_nc(n_cores=NCORES):
    if n_cores not in _NC_CACHE:
        _NC_CACHE[n_cores] = build(n_cores)
    return _NC_CACHE[n_cores]


def make_in_maps(query_tokens, key_tokens, value_tokens, Wk, Wq, Wva, Wal, Wvo):
    qT = np.ascontiguousarray(np.transpose(query_tokens, (0, 2, 1)), np.float32)
    keyT = np.ascontiguousarray(np.transpose(key_tokens, (0, 2, 1)), np.float32)
    valT = np.ascontiguousarray(np.transpose(value_tokens, (0, 2, 1)), np.float32)
    wqT = np.ascontiguousarray(Wq.T, np.float32)
    wkT = np.ascontiguousarray(Wk.T, np.float32)
    wvaT = np.ascontiguousarray(Wva.T, np.float32)
    walT = np.ascontiguousarray(Wal.T).astype(np.float16)
    eye = np.eye(P, dtype=np.float16)
    in_maps = []
    for c in range(NCORES):
        sl = slice(c * KSH, (c + 1) * KSH)
        in_maps.append(
            {
                "wqT": wqT, "wkT": wkT, "wvaT": wvaT, "walT": walT,
                "qT": qT,
                "keyT": np.ascontiguousarray(keyT[:, :, sl]),
                "valT": np.ascontiguousarray(valT[:, :, sl]),
                "eye": eye,
            }
        )
    return in_maps


def kernel(query_tokens, key_tokens, value_tokens, Wk, Wq, Wva, Wal, Wvo):
    args = [np.asarray(a, np.float32) for a in
            (query_tokens, key_tokens, value_tokens, Wk, Wq, Wva, Wal, Wvo)]
    in_maps = make_in_maps(*args)
    nc = _get_nc()
    res = run_bass_kernel_spmd(nc, in_maps, core_ids=list(range(NCORES)))
    total = np.zeros((B, T, Q), np.float32)
    for c in range(NCORES):
        total += res.results[c]["outT"]
    # total is the value-sum transposed [B, T, Q]; apply Wvo on host
    Wvo = np.asarray(args[7], np.float32)
    return np.einsum("ut,btq->bqu", Wvo, total).astype(np.float32)
